# revision 8
# baseline (speedup 1.0000x reference)
"""Trainium2 Bass kernel for nn_Model_25881472926495 (gnn_message_passing).

Model structure (B=32, C=4, N=24 stations, L=8, grid 128x128, HID=128):
  - k=1 nearest-neighbor e2m edges: each station reads ONE grid cell out of
    128*128. Indices depend only on csta/cera (batch-independent), so they are
    computed on host and baked into the DMA access patterns at build time.
  - madis embedding MLP (34->128->128, tanh), two GNN "ex" layers
    (msg: 194->128->128 tanh; upd: 256->128 tanh, 128->128 linear),
    output head (128->128 tanh, 128->24 linear).

Distribution: data-parallel over batch across 8 cores (4 batches/core).
Full era_his/pan_fut shards are shipped to device DRAM; the kernel gathers
only the 24 needed grid columns per (batch, channel) via DMA.

On-chip layout: activations are kept transposed (features on partitions,
batch*station = 96 on the free dim), so every linear layer is a single
matmul out[M,96] = W[K,M].T @ actT[K,96], with W staged in SBUF as-is.
The station/obs feature blocks are DMA'd station-major and flipped with PE
transposes; weight rows are pre-permuted on host to match the on-chip
feature order, which keeps every matmul full-width.
"""

import os
from contextlib import ExitStack

import numpy as np

B, C, N, L = 32, 4, 24, 8
LAT, LON = 128, 128
HID = 128
PRE = 24
NCORES = 8
BS = B // NCORES  # batches per core
COLS = BS * N  # 96 columns (b*24+n)

_CACHE: dict = {}


def _pad_block(w):
    w = np.asarray(w, np.float32)
    out = np.zeros((128, 128), np.float32)
    out[: w.shape[0], : w.shape[1]] = w
    return out


def _pad_bias(b):
    b = np.asarray(b, np.float32)
    out = np.zeros((128,), np.float32)
    out[: b.shape[0]] = b
    return out


def _build_program(li, lo):
    """Build the Bass/Tile program with gather indices baked into DMA APs."""
    import concourse.bacc as bacc
    import concourse.tile as tile
    from concourse import mybir
    from concourse.masks import make_identity

    AF = mybir.ActivationFunctionType
    f32 = mybir.dt.float32

    nc = bacc.Bacc("TRN2", target_bir_lowering=False, debug=False)

    obs_d = nc.dram_tensor("obs", [BS, C, N, L], f32, kind="ExternalInput")
    era_d = nc.dram_tensor("era", [BS, C, LAT, LON + 1, L], f32, kind="ExternalInput")
    pan_d = nc.dram_tensor("pan", [BS, C, LAT, LON, L], f32, kind="ExternalInput")
    posrel_d = nc.dram_tensor("posrel", [4, COLS], f32, kind="ExternalInput")
    wall_d = nc.dram_tensor("wall", [128, 19 * 128], f32, kind="ExternalInput")
    ball_d = nc.dram_tensor("ball", [128, 11], f32, kind="ExternalInput")
    b2bc_d = nc.dram_tensor("b2bc", [COLS, PRE], f32, kind="ExternalInput")
    out_d = nc.dram_tensor("out", [BS, 1, N, PRE], f32, kind="ExternalOutput")

    # weight block column offsets in wall
    (E1X, E1P, E2, M1H1, M1EX1, M1R1, M21, U1H1, U1M1, U21,
     M1H2, M1EX2, M1R2, M22, U1H2, U1M2, U22, O1, O2) = [i * 128 for i in range(19)]
    LAYERS = [
        (M1H1, M1EX1, M1R1, M21, U1H1, U1M1, U21, 2),
        (M1H2, M1EX2, M1R2, M22, U1H2, U1M2, U22, 6),
    ]

    with tile.TileContext(nc) as tc, ExitStack() as ctx:
        const = ctx.enter_context(tc.tile_pool(name="const", bufs=1))
        act = ctx.enter_context(tc.tile_pool(name="act", bufs=8))
        psum = ctx.enter_context(tc.tile_pool(name="psum", bufs=3, space="PSUM"))
        psumt = ctx.enter_context(tc.tile_pool(name="psumt", bufs=1, space="PSUM"))

        wall = const.tile([128, 19 * 128], f32)
        nc.sync.dma_start(wall[:], wall_d.ap())
        ball = const.tile([128, 11], f32)
        nc.sync.dma_start(ball[:], ball_d.ap())
        pos_sb = const.tile([2, COLS], f32)
        nc.sync.dma_start(pos_sb[:], posrel_d.ap()[0:2, :])
        rel_sb = const.tile([2, COLS], f32)
        nc.sync.dma_start(rel_sb[:], posrel_d.ap()[2:4, :])
        b2bc = const.tile([COLS, PRE], f32)
        nc.sync.dma_start(b2bc[:], b2bc_d.ap())

        ident = const.tile([N, N], f32)
        make_identity(nc, ident[:])

        # obs: (b,c,n,l) -> SBUF (n parts, b*32 + c*8 + l), one DMA per batch
        raw_obs = const.tile([N, BS * 32], f32)
        for b in range(BS):
            nc.sync.dma_start(
                raw_obs[:, b * 32 : (b + 1) * 32].rearrange(
                    "n (c l) -> n c l", c=C, l=L
                ),
                obs_d.ap()[b].transpose([1, 0, 2]),
            )

        # station gather: era/pan at the 24 nearest cells
        # SBUF (n parts, t*128 + b*32 + c*8 + l), t=0 era, t=1 pan; per-station
        # dst is one contiguous 128-float run so the DMA AP stays <= 3 dims
        raw_sta = const.tile([N, 2 * BS * 32], f32)
        sta_r = raw_sta[:].rearrange("n (t b c l) -> n t b c l", t=2, b=BS, c=C, l=L)
        for n in range(N):
            iy, ix = int(li[n]), int(lo[n])
            nc.sync.dma_start(
                sta_r[n : n + 1, 0], era_d.ap()[:, :, iy, ix, :].unsqueeze(0)
            )
            nc.sync.dma_start(
                sta_r[n : n + 1, 1], pan_d.ap()[:, :, iy, ix, :].unsqueeze(0)
            )

        # flip station-major blocks to feature-major with PE transposes
        # (transpose outputs must land at PSUM partition 0, so era/pan go
        # side by side in the free dim and are stacked during the copy-out)
        ps_obs = psumt.tile([32, COLS], f32, tag="pso")
        ps_sta = psumt.tile([32, 2 * COLS], f32, tag="pss")
        for b in range(BS):
            nc.tensor.transpose(
                ps_obs[:, b * N : (b + 1) * N], raw_obs[:, b * 32 : (b + 1) * 32], ident[:]
            )
            for t in range(2):
                nc.tensor.transpose(
                    ps_sta[:, t * COLS + b * N : t * COLS + (b + 1) * N],
                    raw_sta[:, t * 128 + b * 32 : t * 128 + (b + 1) * 32],
                    ident[:],
                )
        obsT = act.tile([32, COLS], f32, tag="obsT")
        nc.vector.tensor_copy(obsT[:], ps_obs[:])
        staT = act.tile([64, COLS], f32, tag="staT")
        nc.vector.tensor_copy(staT[0:32, :], ps_sta[:, 0:COLS])
        nc.vector.tensor_copy(staT[32:64, :], ps_sta[:, COLS : 2 * COLS])

        def mm(ps, pairs):
            for i, (lhsT, rhs) in enumerate(pairs):
                nc.tensor.matmul(
                    ps, lhsT, rhs, start=(i == 0), stop=(i == len(pairs) - 1)
                )

        # embedding MLP
        ps = psum.tile([HID, COLS], f32, tag="ps")
        mm(ps[:], [(wall[0:32, E1X : E1X + 128], obsT[:]),
                   (wall[0:2, E1P : E1P + 128], pos_sb[:])])
        h1 = act.tile([HID, COLS], f32, tag="h1")
        nc.scalar.activation(h1[:], ps[:], AF.Tanh, bias=ball[:, 0:1])

        ps = psum.tile([HID, COLS], f32, tag="ps")
        mm(ps[:], [(wall[:, E2 : E2 + 128], h1[:])])
        h = act.tile([HID, COLS], f32, tag="h")
        nc.scalar.activation(h[:], ps[:], AF.Tanh, bias=ball[:, 1:2])

        # two GNN ex layers
        for M1H, M1EX, M1R, M2, U1H, U1M, U2, bb in LAYERS:
            ps = psum.tile([HID, COLS], f32, tag="ps")
            mm(ps[:], [(wall[:, M1H : M1H + 128], h[:]),
                       (wall[0:64, M1EX : M1EX + 128], staT[:]),
                       (wall[0:2, M1R : M1R + 128], rel_sb[:])])
            m1 = act.tile([HID, COLS], f32, tag="m1")
            nc.scalar.activation(m1[:], ps[:], AF.Tanh, bias=ball[:, bb : bb + 1])

            ps = psum.tile([HID, COLS], f32, tag="ps")
            mm(ps[:], [(wall[:, M2 : M2 + 128], m1[:])])
            m = act.tile([HID, COLS], f32, tag="m")
            nc.scalar.activation(m[:], ps[:], AF.Tanh, bias=ball[:, bb + 1 : bb + 2])

            ps = psum.tile([HID, COLS], f32, tag="ps")
            mm(ps[:], [(wall[:, U1H : U1H + 128], h[:]),
                       (wall[:, U1M : U1M + 128], m[:])])
            u = act.tile([HID, COLS], f32, tag="u")
            nc.scalar.activation(u[:], ps[:], AF.Tanh, bias=ball[:, bb + 2 : bb + 3])

            ps = psum.tile([HID, COLS], f32, tag="ps")
            mm(ps[:], [(wall[:, U2 : U2 + 128], u[:])])
            h = act.tile([HID, COLS], f32, tag="h")
            nc.scalar.activation(h[:], ps[:], AF.Identity, bias=ball[:, bb + 3 : bb + 4])

        # output head
        ps = psum.tile([HID, COLS], f32, tag="ps")
        mm(ps[:], [(wall[:, O1 : O1 + 128], h[:])])
        o1 = act.tile([HID, COLS], f32, tag="o1")
        nc.scalar.activation(o1[:], ps[:], AF.Tanh, bias=ball[:, 10:11])

        # final linear in row-major orientation: (96,24) = o1T.T @ Wo2
        ps2 = psumt.tile([COLS, PRE], f32, tag="ps2")
        nc.tensor.matmul(ps2[:], o1[:], wall[:, O2 : O2 + PRE], start=True, stop=True)
        outF = act.tile([COLS, PRE], f32, tag="outF")
        nc.vector.tensor_add(outF[:], ps2[:], b2bc[:])

        nc.sync.dma_start(out_d.ap().rearrange("b o n p -> (b o n) p"), outF[:])

    nc.compile()
    return nc


def _prepare(inputs):
    obs_his = np.ascontiguousarray(np.asarray(inputs["obs_his"], np.float32))
    era_his = np.ascontiguousarray(np.asarray(inputs["era_his"], np.float32))
    pan_fut = np.ascontiguousarray(np.asarray(inputs["pan_fut"], np.float32))
    csta = np.asarray(inputs["csta"], np.float32)
    cera = np.asarray(inputs["cera"], np.float32)
    params = inputs["params"]

    cand = cera[:, :-1, :].reshape(-1, 2)
    d = ((csta[:, None, :] - cand[None, :, :]) ** 2).sum(-1)
    nn = d.argmin(1)
    li, lo = nn // LON, nn % LON

    pos = np.stack([csta[:, 1], csta[:, 0]], -1)  # (N,2)
    gp = np.stack([cand[nn, 1], cand[nn, 0]], -1)
    rel = (pos - gp).astype(np.float32)
    posrel = np.concatenate(
        [np.tile(pos.T.astype(np.float32), (1, BS)), np.tile(rel.T, (1, BS))], 0
    )  # (4, 96)

    # feature permutations: on-chip rows are c*8+l (obs) / t*32+c*8+l (sta);
    # reference feature order is l*4+c (and t*32 + l*4+c for ex features)
    p_cl = np.array([l * C + c for c in range(C) for l in range(L)])
    p_ex = np.array(
        [128 + t * 32 + l * C + c for t in range(2) for c in range(C) for l in range(L)]
    )

    def g(*path):
        node = params
        for k in path:
            node = node[k]
        w, b = node
        return np.asarray(w, np.float32), np.asarray(b, np.float32)

    we1, be1 = g("emb1")
    we2, be2 = g("emb2")
    wo1, bo1 = g("out1")
    wo2, bo2 = g("out2")

    blocks = [_pad_block(we1[p_cl]), _pad_block(we1[32:34]), _pad_block(we2)]
    biases = [_pad_bias(be1), _pad_bias(be2)]
    for lyr in ("ex1", "ex2"):
        wm1, bm1 = g(lyr, "msg1")
        wm2, bm2 = g(lyr, "msg2")
        wu1, bu1 = g(lyr, "upd1")
        wu2, bu2 = g(lyr, "upd2")
        blocks += [
            _pad_block(wm1[0:128]), _pad_block(wm1[p_ex]), _pad_block(wm1[192:194]),
            _pad_block(wm2), _pad_block(wu1[0:128]), _pad_block(wu1[128:256]),
            _pad_block(wu2),
        ]
        biases += [_pad_bias(bm1), _pad_bias(bm2), _pad_bias(bu1), _pad_bias(bu2)]
    blocks += [_pad_block(wo1), _pad_block(wo2)]
    biases += [_pad_bias(bo1)]

    wall = np.ascontiguousarray(np.concatenate(blocks, 1))  # (128, 19*128)
    ball = np.ascontiguousarray(np.stack(biases, 1))  # (128, 11)
    b2bc = np.ascontiguousarray(np.broadcast_to(bo2, (COLS, PRE)).astype(np.float32))

    in_maps = []
    for i in range(NCORES):
        s = slice(i * BS, (i + 1) * BS)
        in_maps.append({
            "obs": obs_his[s], "era": era_his[s], "pan": pan_fut[s],
            "posrel": posrel, "wall": wall, "ball": ball, "b2bc": b2bc,
        })
    return li, lo, in_maps


def _run(inputs, trace=False):
    from concourse.bass_utils import run_bass_kernel_spmd

    li, lo, in_maps = _prepare(inputs)
    key = (li.tobytes(), lo.tobytes())
    if key not in _CACHE:
        _CACHE[key] = _build_program(li, lo)
    nc = _CACHE[key]
    res = run_bass_kernel_spmd(
        nc, in_maps, core_ids=list(range(NCORES)), trace=trace
    )
    out = np.concatenate([r["out"] for r in res.results], 0)
    return out, res


def kernel(**inputs):
    out, _ = _run(inputs, trace=False)
    return out
